# Initial kernel scaffold
#
"""CrossModalAttention TRN2 kernel (v2).

Computation (per batch b):
  Q_m = x_m @ W_m ; K_m = x_m @ W_m^T   (m in {rna, cnv, clinical})
  out  = mean_i( sum_{j!=i} softmax(Q_i K_j^T / 8) @ x_j )

Strategy (v2 — dual-engine exp + concurrent PV halves):
  - Pure data parallel: batch dim 16 sharded 2-per-core across 8 NeuronCores.
  - Tiny Q/K projections precomputed on host; device runs the O(N^2)
    attention.  Scores are computed transposed (ST[m, n]) in PSUM by the PE
    (K=64 contraction, pairs of concurrent row-half matmuls, fp16 operands).
  - exp is SPLIT between two engines running concurrently:
      * ACT: true exp, scale folded, reads score slots straight from PSUM.
      * DVE: Schraudolph fp16 exp — one tensor_scalar (mult+add) producing
        int16 = round(s*(1024*log2e/8) + 1024*(15-c)); bitcast to fp16 IS
        2^(s*log2e/8) with ~1.8% rms sawtooth error that largely cancels
        across the softmax sum (measured end-to-end rel err ~3e-3).
    The per-chunk group->engine pattern is tunable (D_PATTERNS).
  - PV matmul contracts m on the partition dim (full 128-row ingest — the
    PE floor is rhs-ingest bandwidth; score + PV streams already saturate
    the 128 values/cycle input path).
  - Softmax denominator rides as a 65th "feature" column of x_j set to 3.0
    (folds the mean over 3 modalities into the normalization).
  - out^T chunks are PE-transposed back to [n, d], normalized with a
    per-partition reciprocal + one broadcast multiply on DVE, accumulated
    into acc by the (otherwise idle) GPSIMD Pool engine.  Deferred one
    chunk so it never sits between the score matmuls and the exp stream.
  - PSUM: three 2-bank score slots (6) + PV accumulator (1) + transpose
    target (1) = 8 banks.  Score matmuls are emitted THREE groups ahead of
    the exp/PV stream — enough queued PE work to bridge each exp's latency
    so the PE never gaps (gaps reset the 1.2->2.4 GHz p-state ramp).
"""

import os

import numpy as np

import concourse.bass as bass
import concourse.bacc as bacc
import concourse.tile as tile
from concourse import mybir
from concourse.bass_utils import run_bass_kernel_spmd

B, N, D = 16, 2048, 64
NCORES = 8
BPC = B // NCORES  # batches per core
NT = N // 128  # 16 row-tiles of 128
CH = 512  # n-chunk (PSUM bank)
NCH = N // CH  # 4
PAIRS = [(i, j) for i in range(3) for j in range(3) if i != j]
SCALE = 1.0 / 8.0  # 1/sqrt(D)
F32 = mybir.dt.float32
F16 = mybir.dt.float16  # matmul operand dtype: 1 cyc/row
I16 = mybir.dt.int16

NG = 8  # score groups per chunk; group g covers m-tiles (2g, 2g+1)

# Schraudolph constants: int16 = round(s * SCH_A + SCH_B); bitcast fp16.
LOG2E = float(np.log2(np.e))
SCH_C = 0.0580  # minimizes rms rel err of the sawtooth
SCH_A = 1024.0 * LOG2E * SCALE
SCH_B = 1024.0 * (15.0 - SCH_C)

# Which groups go to the DVE (Schraudolph) per chunk, by chunk parity.
# phiD = avg fraction of exp work on DVE.
D_PATTERNS = [(2, 4, 6)]  # phiD = 6/16

_cache = {}
last_results = None  # BassKernelResults of the most recent run (for test.py)


def _build():
    nc = bacc.Bacc()
    qt_d = [
        nc.declare_dram_parameter(f"qt{m}", [BPC, 128, N], F16, isOutput=False)
        for m in range(3)
    ]
    kt_d = [
        nc.declare_dram_parameter(f"kt{m}", [BPC, 128, N], F16, isOutput=False)
        for m in range(3)
    ]
    xo_d = [
        nc.declare_dram_parameter(f"xo{m}", [BPC, 128, NT, D + 1], F16, isOutput=False)
        for m in range(3)
    ]
    id_d = nc.declare_dram_parameter("ident", [128, 128], F32, isOutput=False)
    out_d = nc.declare_dram_parameter("out", [BPC, N, D], F32, isOutput=True)

    from contextlib import ExitStack

    with tile.TileContext(nc) as tc, ExitStack() as ctx:
        singles = ctx.enter_context(tc.tile_pool(name="singles", bufs=1))
        big = ctx.enter_context(tc.tile_pool(name="big", bufs=2))
        work = ctx.enter_context(tc.tile_pool(name="work", bufs=3))
        psum = ctx.enter_context(tc.tile_pool(name="psum", bufs=3, space="PSUM"))

        id_sb = singles.tile([128, 128], F32)
        nc.sync.dma_start(out=id_sb, in_=id_d[:, :])
        # Warm up the ACT engine: absorb the exp-table load and the const
        # bias-AP DMA wait into one early instruction.
        warm = singles.tile([128, 1], F32)
        bias0 = nc.const_aps.scalar_like(0.0, warm[:, 0:1])
        nc.scalar.activation(warm, bias0, mybir.ActivationFunctionType.Exp)

        for b in range(BPC):
            qt_sb, kt_sb, xo_sb = [None] * 3, [None] * 3, [None] * 3
            for m in range(3):
                qt_sb[m] = big.tile([128, N], F16, tag=f"qt{m}", name=f"qt{m}_{b}")
                kt_sb[m] = big.tile([128, N], F16, tag=f"kt{m}", name=f"kt{m}_{b}")
                xo_sb[m] = big.tile(
                    [128, NT, D + 1], F16, tag=f"xo{m}", name=f"xo{m}_{b}"
                )
            # Issue the first pair's ((0,1)) inputs first so compute can start
            # before the remaining loads land.
            for m, t_sb, t_d in (
                (0, qt_sb, qt_d), (1, kt_sb, kt_d), (1, xo_sb, xo_d),
                (1, qt_sb, qt_d), (2, kt_sb, kt_d), (2, xo_sb, xo_d),
                (2, qt_sb, qt_d), (0, kt_sb, kt_d), (0, xo_sb, xo_d),
            ):
                nc.sync.dma_start(out=t_sb[m], in_=t_d[m][b])
            acc = big.tile([128, NT, D], F32, tag="acc", name=f"acc_{b}")

            # Flat schedule of groups; score matmuls are emitted ONE GROUP
            # AHEAD of the exp/PV stream.
            sched = [
                (i, j, c, g) for (i, j) in PAIRS for c in range(NCH)
                for g in range(NG)
            ]
            pending = []  # deferred per-chunk normalize work
            pv_pending = []  # one-group-late PV emission queue

            def flush_pending():
                while pending:
                    pending.pop(0)()

            st_tiles = {}

            def emit_st(idx):
                i, j, c, g = sched[idx]
                stt = psum.tile(
                    [128, 2 * CH], F32, tag="st",
                    name=f"st_{b}_{i}{j}_{c}_{g}",
                )
                st_tiles[idx] = stt
                for p in range(2):
                    t = 2 * g + p
                    h = (t % 2) * 64  # alternate PE row halves -> concurrent
                    nc.tensor.matmul(
                        stt[:, p * CH : (p + 1) * CH],
                        lhsT=kt_sb[j][h : h + 64, t * 128 : (t + 1) * 128],
                        rhs=qt_sb[i][h : h + 64, c * CH : (c + 1) * CH],
                        start=True,
                        stop=True,
                    )

            emit_st(0)
            emit_st(1)
            emit_st(2)
            out_ps = None
            chunk_counter = 0
            for idx, (i, j, c, g) in enumerate(sched):
                if g == 0:
                    out_ps = psum.tile(
                        [D + 1, CH], F32, tag="out", bufs=2,
                        name=f"o_{b}_{i}{j}_{c}",
                    )
                stt = st_tiles.pop(idx)
                dpat = D_PATTERNS[chunk_counter % len(D_PATTERNS)]
                if g in dpat:
                    # DVE Schraudolph exp -> int16, consumed bitcast as fp16
                    ptd = work.tile(
                        [128, 2 * CH], I16, tag="pt", bufs=4,
                        name=f"pt_{b}_{i}{j}_{c}_{g}",
                    )
                    nc.vector.tensor_scalar(
                        out=ptd, in0=stt, scalar1=SCH_A, scalar2=SCH_B,
                        op0=mybir.AluOpType.mult, op1=mybir.AluOpType.add,
                    )
                    ptt = ptd.bitcast(F16)
                else:
                    ptt = work.tile(
                        [128, 2 * CH], F16, tag="pt", bufs=4,
                        name=f"pt_{b}_{i}{j}_{c}_{g}",
                    )
                    nc.scalar.activation(
                        ptt, stt, mybir.ActivationFunctionType.Exp, scale=SCALE
                    )
                # PV matmuls are emitted ONE GROUP LATE so they never
                # wait on a just-finished exp (sem propagation + LDW
                # exposure at the handoff cost ~120ns per group).
                def mk_pv(ptt=ptt, out_ps=out_ps, i=i, j=j, c=c, g=g, b=b):
                    def pv():
                        for p in range(2):
                            t = 2 * g + p
                            nc.tensor.matmul(
                                out_ps,
                                lhsT=(xo_sb[j][:, t, :]),
                                rhs=(ptt[:, p * CH : (p + 1) * CH]),
                                start=(t == 0),
                                stop=(t == NT - 1),
                                skip_group_check=True,
                            )
                        if g == NG - 1:
                            # out_ps rows 0-63 = unnormalized out^T, row 64
                            # = 3*Z.  Copy to SBUF (frees the bank pair);
                            # defer transpose/normalize to flush_pending.
                            osb = work.tile(
                                [D + 1, CH], F32, tag="osb",
                                name=f"osb_{b}_{i}{j}_{c}",
                            )
                            nc.vector.tensor_copy(out=osb, in_=out_ps)
                            mk_normalize(osb, i, j, c)
                    return pv

                pv_pending.append(mk_pv())
                if len(pv_pending) > 1:
                    pv_pending.pop(0)()
                if idx + 3 < len(sched):
                    emit_st(idx + 3)
                if g == 1:
                    flush_pending()
                if g == NG - 1:
                    chunk_counter += 1

                def mk_normalize(osb, i, j, c, b=b, acc=acc):
                    def normalize(osb=osb, b=b, i=i, j=j, c=c, acc=acc):
                        otp = psum.tile(
                            [128, 4, D + 1], F32, tag="st",
                            name=f"otp_{b}_{i}{j}_{c}",
                        )
                        for t in range(4):
                            nc.tensor.transpose(
                                otp[:, t, :],
                                osb[:, t * 128 : (t + 1) * 128],
                                id_sb[0 : D + 1, 0 : D + 1],
                            )
                        rz = work.tile([128, 4], F32, tag="rz", name=f"rz_{b}_{i}{j}_{c}")
                        nc.vector.reciprocal(rz, otp[:, :, D])
                        rzb = rz.unsqueeze(2).broadcast_to([128, 4, D])
                        if (i, j) == PAIRS[0]:
                            # First pair initializes acc directly.
                            nc.vector.tensor_tensor(
                                out=acc[:, c * 4 : (c + 1) * 4, :],
                                in0=otp[:, :, 0:D], in1=rzb,
                                op=mybir.AluOpType.mult,
                            )
                        else:
                            res = work.tile(
                                [128, 4, D], F32, tag="res", name=f"res_{b}_{i}{j}_{c}"
                            )
                            nc.vector.tensor_tensor(
                                out=res, in0=otp[:, :, 0:D], in1=rzb,
                                op=mybir.AluOpType.mult,
                            )
                            # accumulate on the idle Pool engine (SBUF only)
                            nc.gpsimd.tensor_tensor(
                                out=acc[:, c * 4 : (c + 1) * 4, :],
                                in0=acc[:, c * 4 : (c + 1) * 4, :],
                                in1=res,
                                op=mybir.AluOpType.add,
                            )
                        if (i, j) == PAIRS[-1]:
                            # acc chunk is final -- stream it out now so the
                            # kernel tail only carries the last chunk's DMA.
                            nc.sync.dma_start(
                                out=out_d[b].rearrange("(t p) d -> p t d", p=128)[
                                    :, c * 4 : (c + 1) * 4, :
                                ],
                                in_=acc[:, c * 4 : (c + 1) * 4, :],
                            )

                    pending.append(normalize)

            while pv_pending:
                pv_pending.pop(0)()
            flush_pending()
    nc.finalize()  # Bacc: split multi-waits, alloc regs, etc.
    return nc


def _prep(xs, Ws):
    """Host-side input prep: Q/K projections + layout shuffles."""
    qts, kts, xos = [], [], []
    for m in range(3):
        x = np.ascontiguousarray(xs[m], dtype=np.float32)  # [B, N, D]
        W = np.asarray(Ws[m], dtype=np.float32)
        Q = x @ W  # [B, N, D]
        K = x @ W.T
        QT = np.ascontiguousarray(Q.transpose(0, 2, 1))  # [B, D, N]
        KT = np.ascontiguousarray(K.transpose(0, 2, 1))
        qts.append(np.concatenate([QT, QT], axis=1).astype(np.float16))  # [B, 128, N]
        kts.append(np.concatenate([KT, KT], axis=1).astype(np.float16))
        xo = np.full((B, 128, NT, D + 1), 3.0, dtype=np.float16)
        # xo[b, p, t, :64] = x[b, t*128 + p, :]; col 64 stays 3.0 (folds the
        # mean over 3 modalities into the softmax normalization).
        xo[..., :D] = x.reshape(B, NT, 128, D).transpose(0, 2, 1, 3).astype(np.float16)
        xos.append(xo)
    return qts, kts, xos


def kernel(x_rna, x_cnv, x_clinical, W_rna, W_cnv, W_clinical):
    global last_results
    xs = [x_rna, x_cnv, x_clinical]
    Ws = [W_rna, W_cnv, W_clinical]
    qts, kts, xos = _prep(xs, Ws)
    ident = np.eye(128, dtype=np.float32)

    if "nc" not in _cache:
        _cache["nc"] = _build()
    nc = _cache["nc"]

    in_maps = []
    for c in range(NCORES):
        sl = slice(c * BPC, (c + 1) * BPC)
        m = {"ident": ident}
        for mod in range(3):
            m[f"qt{mod}"] = np.ascontiguousarray(qts[mod][sl])
            m[f"kt{mod}"] = np.ascontiguousarray(kts[mod][sl])
            m[f"xo{mod}"] = np.ascontiguousarray(xos[mod][sl])
        in_maps.append(m)

    # The first execution on a freshly-wedged device occasionally fails with
    # NRT_EXEC_UNIT_UNRECOVERABLE; a retry on the reset device succeeds.
    attempt = 0
    while True:
        try:
            last_results = run_bass_kernel_spmd(
                nc,
                in_maps,
                list(range(NCORES)),
                trace=bool(os.environ.get("BASS_TRACE")),
            )
            break
        except Exception:
            attempt += 1
            if attempt > 2:
                raise
    out = np.concatenate([r["out"] for r in last_results.results], axis=0)
    return out



# revision 1
# speedup vs baseline: 1.1730x; 1.1730x over previous
"""CrossModalAttention TRN2 kernel (v2).

Computation (per batch b):
  Q_m = x_m @ W_m ; K_m = x_m @ W_m^T   (m in {rna, cnv, clinical})
  out  = mean_i( sum_{j!=i} softmax(Q_i K_j^T / 8) @ x_j )

Strategy (v2 — dual-engine exp + concurrent PV halves):
  - Pure data parallel: batch dim 16 sharded 2-per-core across 8 NeuronCores.
  - Tiny Q/K projections precomputed on host; device runs the O(N^2)
    attention.  Scores are computed transposed (ST[m, n]) in PSUM by the PE
    (K=64 contraction, pairs of concurrent row-half matmuls, fp16 operands).
  - exp is SPLIT between two engines running concurrently:
      * ACT: true exp, scale folded, reads score slots straight from PSUM.
      * DVE: Schraudolph fp16 exp — one tensor_scalar (mult+add) producing
        int16 = round(s*(1024*log2e/8) + 1024*(15-c)); bitcast to fp16 IS
        2^(s*log2e/8) with ~1.8% rms sawtooth error that largely cancels
        across the softmax sum (measured end-to-end rel err ~3e-3).
    The per-chunk group->engine pattern is tunable (D_PATTERNS).
  - PV matmul contracts m on the partition dim (full 128-row ingest — the
    PE floor is rhs-ingest bandwidth; score + PV streams already saturate
    the 128 values/cycle input path).
  - Softmax denominator rides as a 65th "feature" column of x_j set to 3.0
    (folds the mean over 3 modalities into the normalization).
  - out^T chunks are PE-transposed back to [n, d], normalized with a
    per-partition reciprocal + one broadcast multiply on DVE, accumulated
    into acc by the (otherwise idle) GPSIMD Pool engine.  Deferred one
    chunk so it never sits between the score matmuls and the exp stream.
  - PSUM: three 2-bank score slots (6) + PV accumulator (1) + transpose
    target (1) = 8 banks.  Score matmuls are emitted THREE groups ahead of
    the exp/PV stream — enough queued PE work to bridge each exp's latency
    so the PE never gaps (gaps reset the 1.2->2.4 GHz p-state ramp).
"""

import os

import numpy as np

import concourse.bass as bass
import concourse.bacc as bacc
import concourse.tile as tile
from concourse import mybir
from concourse.bass_utils import run_bass_kernel_spmd

B, N, D = 16, 2048, 64
NCORES = 8
BPC = B // NCORES  # batches per core
NT = N // 128  # 16 row-tiles of 128
CH = 512  # n-chunk (PSUM bank)
NCH = N // CH  # 4
PAIRS = [(i, j) for i in range(3) for j in range(3) if i != j]
SCALE = 1.0 / 8.0  # 1/sqrt(D)
F32 = mybir.dt.float32
F16 = mybir.dt.float16  # matmul operand dtype: 1 cyc/row
I16 = mybir.dt.int16

NG = 8  # score groups per chunk; group g covers m-tiles (2g, 2g+1)

# Schraudolph constants: int16 = round(s * SCH_A + SCH_B); bitcast fp16.
LOG2E = float(np.log2(np.e))
SCH_C = 0.0580  # minimizes rms rel err of the sawtooth
SCH_A = 1024.0 * LOG2E * SCALE
SCH_B = 1024.0 * (15.0 - SCH_C)

# Which groups go to the DVE (Schraudolph) per chunk, by chunk parity.
# phiD = avg fraction of exp work on DVE.
D_PATTERNS = [(2, 4, 6)]  # phiD = 6/16

_cache = {}
last_results = None  # BassKernelResults of the most recent run (for test.py)


def _build():
    nc = bacc.Bacc()
    qt_d = [
        nc.declare_dram_parameter(f"qt{m}", [BPC, 128, N], F16, isOutput=False)
        for m in range(3)
    ]
    kt_d = [
        nc.declare_dram_parameter(f"kt{m}", [BPC, 128, N], F16, isOutput=False)
        for m in range(3)
    ]
    xo_d = [
        nc.declare_dram_parameter(f"xo{m}", [BPC, 128, NT, D + 1], F16, isOutput=False)
        for m in range(3)
    ]
    id_d = nc.declare_dram_parameter("ident", [128, 128], F32, isOutput=False)
    out_d = nc.declare_dram_parameter("out", [BPC, N, D], F32, isOutput=True)

    from contextlib import ExitStack

    with tile.TileContext(nc) as tc, ExitStack() as ctx:
        singles = ctx.enter_context(tc.tile_pool(name="singles", bufs=1))
        big = ctx.enter_context(tc.tile_pool(name="big", bufs=2))
        work = ctx.enter_context(tc.tile_pool(name="work", bufs=3))
        psum = ctx.enter_context(tc.tile_pool(name="psum", bufs=3, space="PSUM"))

        id_sb = singles.tile([128, 128], F32)
        nc.sync.dma_start(out=id_sb, in_=id_d[:, :])
        # Warm up the ACT engine: absorb the exp-table load and the const
        # bias-AP DMA wait into one early instruction.
        warm = singles.tile([128, 1], F32)
        bias0 = nc.const_aps.scalar_like(0.0, warm[:, 0:1])
        nc.scalar.activation(warm, bias0, mybir.ActivationFunctionType.Exp)

        for b in range(BPC):
            qt_sb, kt_sb, xo_sb = [None] * 3, [None] * 3, [None] * 3
            for m in range(3):
                qt_sb[m] = big.tile([128, N], F16, tag=f"qt{m}", name=f"qt{m}_{b}")
                kt_sb[m] = big.tile([128, N], F16, tag=f"kt{m}", name=f"kt{m}_{b}")
                xo_sb[m] = big.tile(
                    [128, NT, D + 1], F16, tag=f"xo{m}", name=f"xo{m}_{b}"
                )
            # Issue the first pair's ((0,1)) inputs first so compute can start
            # before the remaining loads land.
            for m, t_sb, t_d in (
                (0, qt_sb, qt_d), (1, kt_sb, kt_d), (1, xo_sb, xo_d),
                (1, qt_sb, qt_d), (2, kt_sb, kt_d), (2, xo_sb, xo_d),
                (2, qt_sb, qt_d), (0, kt_sb, kt_d), (0, xo_sb, xo_d),
            ):
                nc.sync.dma_start(out=t_sb[m], in_=t_d[m][b])
            acc = big.tile([128, NT, D], F32, tag="acc", name=f"acc_{b}")

            # Flat schedule of groups; score matmuls are emitted ONE GROUP
            # AHEAD of the exp/PV stream.
            sched = [
                (i, j, c, g) for (i, j) in PAIRS for c in range(NCH)
                for g in range(NG)
            ]
            pending = []  # deferred per-chunk normalize work
            pv_pending = []  # one-group-late PV emission queue

            def flush_pending():
                while pending:
                    pending.pop(0)()

            st_tiles = {}

            def emit_st(idx):
                i, j, c, g = sched[idx]
                stt = psum.tile(
                    [128, 2 * CH], F32, tag="st",
                    name=f"st_{b}_{i}{j}_{c}_{g}",
                )
                st_tiles[idx] = stt
                for p in range(2):
                    t = 2 * g + p
                    h = (t % 2) * 64  # alternate PE row halves -> concurrent
                    nc.tensor.matmul(
                        stt[:, p * CH : (p + 1) * CH],
                        lhsT=kt_sb[j][h : h + 64, t * 128 : (t + 1) * 128],
                        rhs=qt_sb[i][h : h + 64, c * CH : (c + 1) * CH],
                        start=True,
                        stop=True,
                    )

            emit_st(0)
            emit_st(1)
            emit_st(2)
            out_ps = None
            chunk_counter = 0
            for idx, (i, j, c, g) in enumerate(sched):
                if g == 0:
                    out_ps = psum.tile(
                        [D + 1, CH], F32, tag="out", bufs=2,
                        name=f"o_{b}_{i}{j}_{c}",
                    )
                stt = st_tiles.pop(idx)
                dpat = D_PATTERNS[chunk_counter % len(D_PATTERNS)]
                if g in dpat:
                    # DVE Schraudolph exp -> int16, consumed bitcast as fp16
                    ptd = work.tile(
                        [128, 2 * CH], I16, tag="pt", bufs=4,
                        name=f"pt_{b}_{i}{j}_{c}_{g}",
                    )
                    nc.vector.tensor_scalar(
                        out=ptd, in0=stt, scalar1=SCH_A, scalar2=SCH_B,
                        op0=mybir.AluOpType.mult, op1=mybir.AluOpType.add,
                    )
                    ptt = ptd.bitcast(F16)
                else:
                    ptt = work.tile(
                        [128, 2 * CH], F16, tag="pt", bufs=4,
                        name=f"pt_{b}_{i}{j}_{c}_{g}",
                    )
                    nc.scalar.activation(
                        ptt, stt, mybir.ActivationFunctionType.Exp, scale=SCALE
                    )
                # PV matmuls are emitted ONE GROUP LATE so they never
                # wait on a just-finished exp (sem propagation + LDW
                # exposure at the handoff cost ~120ns per group).
                def mk_pv(ptt=ptt, out_ps=out_ps, i=i, j=j, c=c, g=g, b=b):
                    def pv():
                        for p in range(2):
                            t = 2 * g + p
                            nc.tensor.matmul(
                                out_ps,
                                lhsT=(xo_sb[j][:, t, :]),
                                rhs=(ptt[:, p * CH : (p + 1) * CH]),
                                start=(t == 0),
                                stop=(t == NT - 1),
                                skip_group_check=True,
                            )
                        if g == NG - 1:
                            # out_ps rows 0-63 = unnormalized out^T, row 64
                            # = 3*Z.  Copy to SBUF (frees the bank pair);
                            # defer transpose/normalize to flush_pending.
                            osb = work.tile(
                                [D + 1, CH], F32, tag="osb",
                                name=f"osb_{b}_{i}{j}_{c}",
                            )
                            nc.vector.tensor_copy(out=osb, in_=out_ps)
                            mk_normalize(osb, i, j, c)
                    return pv

                pv_pending.append(mk_pv())
                if len(pv_pending) > 1:
                    pv_pending.pop(0)()
                if idx + 3 < len(sched):
                    emit_st(idx + 3)
                if g == 1:
                    flush_pending()
                if g == NG - 1:
                    chunk_counter += 1

                def mk_normalize(osb, i, j, c, b=b, acc=acc):
                    def normalize(osb=osb, b=b, i=i, j=j, c=c, acc=acc):
                        otp = psum.tile(
                            [128, 4, D + 1], F32, tag="st",
                            name=f"otp_{b}_{i}{j}_{c}",
                        )
                        for t in range(4):
                            nc.tensor.transpose(
                                otp[:, t, :],
                                osb[:, t * 128 : (t + 1) * 128],
                                id_sb[0 : D + 1, 0 : D + 1],
                            )
                        rz = work.tile([128, 4], F32, tag="rz", name=f"rz_{b}_{i}{j}_{c}")
                        nc.vector.reciprocal(rz, otp[:, :, D])
                        rzb = rz.unsqueeze(2).broadcast_to([128, 4, D])
                        if (i, j) == PAIRS[0]:
                            # First pair initializes acc directly.
                            nc.vector.tensor_tensor(
                                out=acc[:, c * 4 : (c + 1) * 4, :],
                                in0=otp[:, :, 0:D], in1=rzb,
                                op=mybir.AluOpType.mult,
                            )
                        else:
                            res = work.tile(
                                [128, 4, D], F32, tag="res", name=f"res_{b}_{i}{j}_{c}"
                            )
                            nc.vector.tensor_tensor(
                                out=res, in0=otp[:, :, 0:D], in1=rzb,
                                op=mybir.AluOpType.mult,
                            )
                            # accumulate on the idle Pool engine (SBUF only)
                            nc.gpsimd.tensor_tensor(
                                out=acc[:, c * 4 : (c + 1) * 4, :],
                                in0=acc[:, c * 4 : (c + 1) * 4, :],
                                in1=res,
                                op=mybir.AluOpType.add,
                            )
                        if (i, j) == PAIRS[-1]:
                            # acc chunk is final -- stream it out now so the
                            # kernel tail only carries the last chunk's DMA.
                            nc.sync.dma_start(
                                out=out_d[b].rearrange("(t p) d -> p t d", p=128)[
                                    :, c * 4 : (c + 1) * 4, :
                                ],
                                in_=acc[:, c * 4 : (c + 1) * 4, :],
                            )

                    pending.append(normalize)

            while pv_pending:
                pv_pending.pop(0)()
            flush_pending()
    nc.finalize()  # Bacc: split multi-waits, alloc regs, etc.
    return nc


def _prep(xs, Ws):
    """Host-side input prep: Q/K projections + layout shuffles."""
    qts, kts, xos = [], [], []
    for m in range(3):
        x = np.ascontiguousarray(xs[m], dtype=np.float32)  # [B, N, D]
        W = np.asarray(Ws[m], dtype=np.float32)
        Q = x @ W  # [B, N, D]
        K = x @ W.T
        QT = np.ascontiguousarray(Q.transpose(0, 2, 1))  # [B, D, N]
        KT = np.ascontiguousarray(K.transpose(0, 2, 1))
        qts.append(np.concatenate([QT, QT], axis=1).astype(np.float16))  # [B, 128, N]
        kts.append(np.concatenate([KT, KT], axis=1).astype(np.float16))
        xo = np.full((B, 128, NT, D + 1), 3.0, dtype=np.float16)
        # xo[b, p, t, :64] = x[b, t*128 + p, :]; col 64 stays 3.0 (folds the
        # mean over 3 modalities into the softmax normalization).
        xo[..., :D] = x.reshape(B, NT, 128, D).transpose(0, 2, 1, 3).astype(np.float16)
        xos.append(xo)
    return qts, kts, xos


def kernel(x_rna, x_cnv, x_clinical, W_rna, W_cnv, W_clinical):
    global last_results
    xs = [x_rna, x_cnv, x_clinical]
    Ws = [W_rna, W_cnv, W_clinical]
    qts, kts, xos = _prep(xs, Ws)
    ident = np.eye(128, dtype=np.float32)

    if "nc" not in _cache:
        _cache["nc"] = _build()
    nc = _cache["nc"]

    in_maps = []
    for c in range(NCORES):
        sl = slice(c * BPC, (c + 1) * BPC)
        m = {"ident": ident}
        for mod in range(3):
            m[f"qt{mod}"] = np.ascontiguousarray(qts[mod][sl])
            m[f"kt{mod}"] = np.ascontiguousarray(kts[mod][sl])
            m[f"xo{mod}"] = np.ascontiguousarray(xos[mod][sl])
        in_maps.append(m)

    # The first execution on a freshly-wedged device occasionally fails with
    # NRT_EXEC_UNIT_UNRECOVERABLE; a retry on the reset device succeeds.
    attempt = 0
    while True:
        try:
            last_results = run_bass_kernel_spmd(
                nc,
                in_maps,
                list(range(NCORES)),
                trace=bool(os.environ.get("BASS_TRACE")),
            )
            break
        except Exception:
            attempt += 1
            if attempt > 2:
                raise
    out = np.concatenate([r["out"] for r in last_results.results], axis=0)
    return out

